# revision 21
# baseline (speedup 1.0000x reference)
"""Trainium2 Bass kernel for RecursiveMamba130M (fp16, v-recursion pipeline).

Math: the complex SSM state collapses to a depthwise convolution over
reasoning loops:
  y_i[t, f] = sum_{k<=i} G_{i-k}[f] * u_k[t, f],   u_k = h_k @ W_in^T
  G_m[f] = sum_s (CrBr - CiBi) cos(m th) - (CrBi + CiBr) sin(m th)

Key restructure: h_{i+1} = a*z_i + b*h_i + step_{i+1} with per-token
scalars a = rs_z*rs_w, b = rs_w.  Right-multiplying by W_in^T:
  u_{i+1} = a*(z_i @ W_in^T) + b*u_i + su_{i+1},   su = step @ W_in^T
so MM1 of loop i+1 becomes v = z_i @ W_in^T, which depends only on z_i
(NOT on the rmsnorm scalars).  The PE therefore flows
MM2 -> transpose(z) -> MM1' -> transpose(y) -> MM2 with no norm stall;
the rmsnorm chain, u/h recovery and acc updates overlap under MM1'/MM2.

rmsnorm sums via the identity
  sum w^2 = rs_z*(rs_z*sum z^2 + 2*sum z*h) + sum h^2.

Sharding: data-parallel over sequence: 128 tokens per core, no
collectives; weights replicated.  fp16 data / fp32 PSUM + scalars.
"""

import numpy as np

import concourse.bass as bass
import concourse.tile as tile
from concourse.bacc import Bacc
from concourse import masks, mybir
from concourse.bass_utils import run_bass_kernel_spmd

T = 128          # tokens per core
D = 768          # d_model
F = 1536         # 2 * d_model
NL = 4           # reasoning loops
NCORES = 8
EPS = 1e-6

f16 = mybir.dt.float16
f32 = mybir.dt.float32
AL = mybir.AluOpType
AF = mybir.ActivationFunctionType

_CACHE = {}


def build_nc():
    nc = Bacc()
    h0_d = nc.dram_tensor("h0", [T, D], f16, kind="ExternalInput")
    h0T_d = nc.dram_tensor("h0T", [D, T], f16, kind="ExternalInput")
    winT_d = nc.dram_tensor("winT", [D, F], f16, kind="ExternalInput")
    woutT_d = nc.dram_tensor("woutT", [F, D], f16, kind="ExternalInput")
    g4_d = nc.dram_tensor("g4", [NL, F], f16, kind="ExternalInput")
    s4_d = nc.dram_tensor("s4", [NL, D], f16, kind="ExternalInput")
    su4_d = nc.dram_tensor("su4", [NL, F], f16, kind="ExternalInput")
    out_d = nc.dram_tensor("x_out", [T, D], f32, kind="ExternalOutput")

    with tile.TileContext(nc) as tc:
        with (
            tc.tile_pool(name="wpool", bufs=1) as wpool,
            tc.tile_pool(name="apool", bufs=1) as apool,
            tc.tile_pool(name="work", bufs=2) as work,
            tc.tile_pool(name="scal", bufs=1) as scal,
            tc.tile_pool(name="ps_t", bufs=1, space="PSUM") as ps_t,
            tc.tile_pool(name="ps_v", bufs=1, space="PSUM") as ps_v,
            tc.tile_pool(name="ps_z", bufs=1, space="PSUM") as ps_z,
        ):
            # ---------- constants ----------
            ident = wpool.tile([128, 128], f16, tag="ident")
            masks.make_identity(nc, ident[:])
            ones1 = wpool.tile([1, 128], f16, tag="ones1")
            nc.vector.memset(ones1[:], 1.0)
            eps_t = wpool.tile([T, 1], f32, tag="eps_t")
            nc.vector.memset(eps_t[:], EPS)

            # ---------- small DMAs first (sync queue) ----------
            g_rows = []
            for m in range(NL):
                gr = work.tile([1, F], f16, tag="g_row", bufs=4, name=f"g_row{m}")
                nc.sync.dma_start(gr[:], g4_d[m:m + 1, :])
                g_rows.append(gr)
            s_rows = {}
            for i in range(1, NL):
                sr = work.tile([1, D], f16, tag="s_row", bufs=3, name=f"s_row{i}")
                nc.sync.dma_start(sr[:], s4_d[i:i + 1, :])
                s_rows[i] = sr
            su_rows = {}
            for i in range(1, NL):
                sr = work.tile([1, F], f16, tag="su_row", bufs=3, name=f"su_row{i}")
                nc.sync.dma_start(sr[:], su4_d[i:i + 1, :])
                su_rows[i] = sr
            h = work.tile([T, D], f16, tag="h", bufs=2)
            nc.sync.dma_start(h[:], h0_d[:, :])
            hT0 = work.tile([T, D], f16, tag="hT0", bufs=1)
            for k in range(6):
                nc.sync.dma_start(hT0[:, 128 * k:128 * (k + 1)],
                                  h0T_d[128 * k:128 * (k + 1), :])

            winT_sb = []
            for k in range(6):
                wt = wpool.tile([128, F], f16, tag=f"winT{k}")
                nc.sync.dma_start(wt[:], winT_d[128 * k:128 * (k + 1), :])
                winT_sb.append(wt)
            woutT_sb = []
            for c in range(12):
                wt = wpool.tile([128, D], f16, tag=f"woutT{c}")
                nc.sync.dma_start(wt[:], woutT_d[128 * c:128 * (c + 1), :])
                woutT_sb.append(wt)

            SL = [slice(512 * n, 512 * (n + 1)) for n in range(3)]

            # ---------- broadcast tiles via PE ones-trick ----------
            def bcast_f(row, tag):  # [1, F] -> [128, F]
                sb = wpool.tile([T, F], f16, tag=tag)
                for n in range(3):
                    ps = ps_v.tile([T, 512], f32, tag=f"v{n}", name=f"bc_{tag}{n}")
                    nc.tensor.matmul(ps[:, :], ones1[:, :], row[:, SL[n]],
                                     start=True, stop=True)
                    nc.scalar.copy(sb[:, SL[n]], ps[:, :])
                return sb

            def bcast_d(row, tag):  # [1, D] -> [128, D]
                ps = ps_z.tile([T, D], f32, tag="z")
                for off, nn in ((0, 512), (512, 256)):
                    nc.tensor.matmul(ps[:, off:off + nn], ones1[:, :],
                                     row[:, off:off + nn], start=True, stop=True)
                sb = wpool.tile([T, D], f16, tag=tag)
                nc.scalar.copy(sb[:], ps[:])
                return sb

            Gb = [bcast_f(g_rows[m], f"Gb{m}") for m in range(NL)]
            SU = {i: bcast_f(su_rows[i], f"SU{i}") for i in range(1, NL)}
            Sb = {i: bcast_d(s_rows[i], f"Sb{i}") for i in range(1, NL)}

            accs = {j: apool.tile([T, F], f16, tag=f"acc{j}", name=f"acc{j}")
                    for j in (1, 2, 3)}
            rs_w_p = a_p = None
            u_prev = None
            zT_sb = None

            # Warm-keeper: dummy matmuls into a spare PSUM bank keep the PE
            # HAM clock-gate at 8/8 through dependency handoffs (PE transposes
            # do not count as HAM-busy, and any >~3.4us idle re-throttles to
            # 1.2 GHz).  FIFO placement makes them run exactly in the stalls.
            dummy_ps = ps_t.tile([T, 512], f32, tag="dummy")

            def warm(n_mm):
                for _ in range(n_mm):
                    nc.tensor.matmul(dummy_ps[:, :], ones1[:, :],
                                     g_rows[0][:, 0:512], start=True,
                                     stop=True, skip_group_check=True)

            for i in range(NL):
                first, last = i == 0, i == NL - 1

                # ---- T1_n = b*u_prev + SU_i (early window ops, old scalars)
                t1s = []
                if not first:
                    for n in range(3):
                        t1 = work.tile([T, 512], f16, tag=f"t1_{n}", bufs=2,
                                       name=f"t1_{n}")
                        nc.vector.scalar_tensor_tensor(
                            out=t1[:], in0=u_prev[:, SL[n]],
                            scalar=rs_w_p[:, :], in1=SU[i][:, SL[n]],
                            op0=AL.mult, op1=AL.add)
                        t1s.append(t1)

                # ---- MM1: v_i = (h_0 or z_{i-1}) @ W_in^T ----
                v_ps = [ps_v.tile([T, 512], f32, tag=f"v{n}", name=f"v{n}_{i}")
                        for n in range(3)]
                lhsT = hT0 if first else zT_sb
                if first:
                    for k in range(6):      # k-outer: winT DMA arrival order
                        for n in range(3):
                            nc.tensor.matmul(
                                v_ps[n][:, :], lhsT[:, 128 * k:128 * (k + 1)],
                                winT_sb[k][:, SL[n]],
                                start=(k == 0), stop=(k == 5))
                        warm(2)             # stay HAM-busy during winT DMA
                else:
                    for n in range(3):      # n-outer: slices finish early
                        for k in range(6):
                            nc.tensor.matmul(
                                v_ps[n][:, :], lhsT[:, 128 * k:128 * (k + 1)],
                                winT_sb[k][:, SL[n]],
                                start=(k == 0), stop=(k == 5))
                    warm(4)                 # cover the y-combine handoff

                # ---- per-slice: u and y; then yT transposes ----
                u_cur = work.tile([T, F], f16, tag="u", bufs=3, name=f"u{i}")
                y = work.tile([T, F], f16, tag="y", bufs=1)
                yT_ps = ps_t.tile([T, F], f16, tag="t")
                for n in range(3):
                    sl = SL[n]
                    if first:
                        # u = v (copy for acc updates, ACT); y = G0*u
                        nc.scalar.copy(u_cur[:, sl], v_ps[n][:, :])
                        nc.vector.tensor_mul(y[:, sl], v_ps[n][:, :],
                                             Gb[0][:, sl])
                    else:
                        # u = a*v + T1   (critical)
                        nc.vector.scalar_tensor_tensor(
                            out=u_cur[:, sl], in0=v_ps[n][:, :],
                            scalar=a_p[:, :], in1=t1s[n][:],
                            op0=AL.mult, op1=AL.add)
                        # y = G0*u + acc_i
                        ym = work.tile([T, 512], f16, tag=f"ym_{n}", bufs=2,
                                       name=f"ym_{n}")
                        nc.vector.tensor_mul(ym[:], u_cur[:, sl], Gb[0][:, sl])
                        nc.vector.tensor_add(y[:, sl], ym[:], accs[i][:, sl])
                    for c in range(4 * n, 4 * n + 4):
                        nc.tensor.transpose(
                            yT_ps[:, 128 * c:128 * (c + 1)],
                            y[:, 128 * c:128 * (c + 1)], ident[:])

                yT_sb = work.tile([T, F], f16, tag="yT_sb", bufs=1)
                for n in range(3):
                    nc.scalar.copy(yT_sb[:, SL[n]], yT_ps[:, SL[n]])

                # ---- eager acc updates for future loops (gpsimd, off-path) ----
                for j in range(i + 1, NL):
                    m = j - i
                    if first:
                        nc.gpsimd.tensor_mul(accs[j][:], u_cur[:], Gb[m][:])
                    else:
                        tmp_a = work.tile([T, F], f16, tag="tmp_a", bufs=2)
                        nc.gpsimd.tensor_mul(tmp_a[:], u_cur[:], Gb[m][:])
                        nc.gpsimd.tensor_add(accs[j][:], accs[j][:], tmp_a[:])

                # ---- MM2: z = y @ out_proj^T ----
                z_ps = ps_z.tile([T, D], f32, tag="z")
                for c in range(12):
                    for off, nn in ((0, 512), (512, 256)):
                        nc.tensor.matmul(
                            z_ps[:, off:off + nn],
                            yT_sb[:, 128 * c:128 * (c + 1)],
                            woutT_sb[c][:, off:off + nn],
                            start=(c == 0), stop=(c == 11))
                    if first:
                        warm(1)             # stay HAM-busy during woutT DMA
                warm(6)                     # cover the z->zT handoff

                # ---- z -> SBUF f16; zT (next loop's MM1 stationary) ----
                z_sb = work.tile([T, D], f16, tag="z_sb", bufs=2)
                nc.scalar.copy(z_sb[:, 0:384], z_ps[:, 0:384])
                nc.scalar.copy(z_sb[:, 384:768], z_ps[:, 384:768])
                if not last:
                    zT_ps = ps_t.tile([T, D], f16, tag="t")
                    for k in range(6):
                        nc.tensor.transpose(
                            zT_ps[:, 128 * k:128 * (k + 1)],
                            z_sb[:, 128 * k:128 * (k + 1)], ident[:])
                    zT_sb = work.tile([T, D], f16, tag="zT_sb", bufs=2)
                    nc.scalar.copy(zT_sb[:, 0:384], zT_ps[:, 0:384])
                    nc.scalar.copy(zT_sb[:, 384:768], zT_ps[:, 384:768])

                # ---- rmsnorm chain (direct form; deadlines are loose:
                # rs_w/a feed the NEXT loop's window ops only) ----
                ss_z = scal.tile([T, 1], f32, tag="ss_z", bufs=2)
                sq_scr = work.tile([T, D], f16, tag="scr", bufs=3)
                nc.scalar.activation(sq_scr[:], z_ps[:], AF.Square,
                                     accum_out=ss_z[:])
                sq_z = scal.tile([T, 1], f32, tag="sq_z", bufs=2)
                nc.scalar.activation(sq_z[:], ss_z[:], AF.Sqrt,
                                     bias=eps_t[:, :], scale=1.0 / D)
                rs_z = scal.tile([T, 1], f32, tag="rs_z", bufs=2)
                nc.vector.reciprocal(rs_z[:], sq_z[:])

                # w = z*rs_z + h
                last_dt = f32 if last else f16
                w = work.tile([T, D], last_dt, tag="w", bufs=2, name=f"w{i}")
                nc.vector.scalar_tensor_tensor(
                    out=w[:], in0=z_sb[:], scalar=rs_z[:, :], in1=h[:],
                    op0=AL.mult, op1=AL.add)
                ss_w = scal.tile([T, 1], f32, tag="ss_w", bufs=2)
                sq_scr2 = work.tile([T, D], f16, tag="scr", bufs=3)
                nc.scalar.activation(sq_scr2[:], w[:], AF.Square,
                                     accum_out=ss_w[:])
                sq_w = scal.tile([T, 1], f32, tag="sq_w", bufs=2)
                nc.scalar.activation(sq_w[:], ss_w[:], AF.Sqrt,
                                     bias=eps_t[:, :], scale=1.0 / D)
                rs_w = scal.tile([T, 1], f32, tag="rs_w", bufs=2)
                nc.vector.reciprocal(rs_w[:], sq_w[:])
                a_t = scal.tile([T, 1], f32, tag="a_t", bufs=2)
                nc.vector.tensor_mul(a_t[:], rs_z[:], rs_w[:])

                # ---- h update: h' = w*rs_w + Sb_{i+1} ----
                if not last:
                    h_next = work.tile([T, D], f16, tag="h", bufs=2)
                    nc.vector.scalar_tensor_tensor(
                        out=h_next[:], in0=w[:], scalar=rs_w[:, :],
                        in1=Sb[i + 1][:], op0=AL.mult, op1=AL.add)
                    h = h_next
                else:
                    x_out = work.tile([T, D], f32, tag="x_out", bufs=1)
                    nc.vector.tensor_scalar_mul(x_out[:], w[:], rs_w[:, :])
                    nc.sync.dma_start(out_d[:, :], x_out[:])

                rs_w_p, a_p = rs_w, a_t
                u_prev = u_cur

    nc.compile()
    return nc


def _host_prep(x, in_proj_base, lora_A, lora_B, A_theta, B_real, B_imag,
               C_real, C_imag, out_proj_w, step_emb):
    W_in = in_proj_base.astype(np.float64) + 2.0 * (
        lora_B.astype(np.float64) @ lora_A.astype(np.float64))
    winT = np.ascontiguousarray(W_in.T).astype(np.float16)
    woutT = np.ascontiguousarray(out_proj_w.T).astype(np.float16)

    th = A_theta.astype(np.float64)
    P = (C_real.astype(np.float64) * B_real.astype(np.float64)
         - C_imag.astype(np.float64) * B_imag.astype(np.float64))
    Q = (C_real.astype(np.float64) * B_imag.astype(np.float64)
         + C_imag.astype(np.float64) * B_real.astype(np.float64))
    g4 = np.stack([
        (P * np.cos(m * th) - Q * np.sin(m * th)).sum(-1).reshape(-1)
        for m in range(NL)
    ]).astype(np.float16)                                   # [4, 1536]
    s4 = np.ascontiguousarray(step_emb).astype(np.float16)  # [4, 768]
    su4 = (step_emb.astype(np.float64) @ W_in.T).astype(np.float16)  # [4,1536]

    h0 = (x[0].astype(np.float64) + step_emb[0].astype(np.float64)
          ).astype(np.float16)                              # [1024, 768]
    h0T = np.ascontiguousarray(h0.T)                        # [768, 1024]
    ssh0 = (h0.astype(np.float32) ** 2).sum(-1, keepdims=True)  # [1024, 1]
    return winT, woutT, g4, s4, su4, h0, h0T, ssh0


def kernel(x, in_proj_base, lora_A, lora_B, A_theta, B_real, B_imag,
           C_real, C_imag, out_proj_w, mixer_norm_w, loop_norm_w, step_emb,
           _trace=False):
    winT, woutT, g4, s4, su4, h0, h0T, ssh0 = _host_prep(
        np.asarray(x, dtype=np.float32), np.asarray(in_proj_base),
        np.asarray(lora_A), np.asarray(lora_B), np.asarray(A_theta),
        np.asarray(B_real), np.asarray(B_imag), np.asarray(C_real),
        np.asarray(C_imag), np.asarray(out_proj_w), np.asarray(step_emb))
    # mixer_norm_w / loop_norm_w are ones per the problem spec; rmsnorm weight
    # multiplies are identity and omitted on device.

    if "nc" not in _CACHE:
        _CACHE["nc"] = build_nc()
    nc = _CACHE["nc"]

    shared = {"winT": winT, "woutT": woutT, "g4": g4, "s4": s4, "su4": su4}
    in_maps = [
        {**shared,
         "h0": np.ascontiguousarray(h0[T * c:T * (c + 1), :]),
         "h0T": np.ascontiguousarray(h0T[:, T * c:T * (c + 1)])}
        for c in range(NCORES)
    ]
    res = run_bass_kernel_spmd(nc, in_maps, list(range(NCORES)), trace=_trace)
    out = np.concatenate(
        [np.asarray(res.results[c]["x_out"]) for c in range(NCORES)], axis=0)
    if _trace:
        _CACHE["last_result"] = res
    return out[None, :, :].astype(np.float32)


# revision 22
# speedup vs baseline: 1.0423x; 1.0423x over previous
"""Trainium2 Bass kernel for RecursiveMamba130M (fp16, v-recursion pipeline).

Math: the complex SSM state collapses to a depthwise convolution over
reasoning loops:
  y_i[t, f] = sum_{k<=i} G_{i-k}[f] * u_k[t, f],   u_k = h_k @ W_in^T
  G_m[f] = sum_s (CrBr - CiBi) cos(m th) - (CrBi + CiBr) sin(m th)

Key restructure: h_{i+1} = a*z_i + b*h_i + step_{i+1} with per-token
scalars a = rs_z*rs_w, b = rs_w.  Right-multiplying by W_in^T:
  u_{i+1} = a*(z_i @ W_in^T) + b*u_i + su_{i+1},   su = step @ W_in^T
so MM1 of loop i+1 becomes v = z_i @ W_in^T, which depends only on z_i
(NOT on the rmsnorm scalars).  The PE therefore flows
MM2 -> transpose(z) -> MM1' -> transpose(y) -> MM2 with no norm stall;
the rmsnorm chain, u/h recovery and acc updates overlap under MM1'/MM2.

rmsnorm sums via the identity
  sum w^2 = rs_z*(rs_z*sum z^2 + 2*sum z*h) + sum h^2.

Sharding: data-parallel over sequence: 128 tokens per core, no
collectives; weights replicated.  fp16 data / fp32 PSUM + scalars.
"""

import numpy as np

import concourse.bass as bass
import concourse.tile as tile
from concourse.bacc import Bacc
from concourse import masks, mybir
from concourse.bass_utils import run_bass_kernel_spmd

T = 128          # tokens per core
D = 768          # d_model
F = 1536         # 2 * d_model
NL = 4           # reasoning loops
NCORES = 8
EPS = 1e-6

f16 = mybir.dt.float16
f32 = mybir.dt.float32
AL = mybir.AluOpType
AF = mybir.ActivationFunctionType

_CACHE = {}


def build_nc():
    nc = Bacc()
    h0_d = nc.dram_tensor("h0", [T, D], f16, kind="ExternalInput")
    h0T_d = nc.dram_tensor("h0T", [D, T], f16, kind="ExternalInput")
    winT_d = nc.dram_tensor("winT", [D, F], f16, kind="ExternalInput")
    woutT_d = nc.dram_tensor("woutT", [F, D], f16, kind="ExternalInput")
    g4_d = nc.dram_tensor("g4", [NL, F], f16, kind="ExternalInput")
    s4_d = nc.dram_tensor("s4", [NL, D], f16, kind="ExternalInput")
    su4_d = nc.dram_tensor("su4", [NL, F], f16, kind="ExternalInput")
    out_d = nc.dram_tensor("x_out", [T, D], f32, kind="ExternalOutput")

    with tile.TileContext(nc) as tc:
        with (
            tc.tile_pool(name="wpool", bufs=1) as wpool,
            tc.tile_pool(name="apool", bufs=1) as apool,
            tc.tile_pool(name="work", bufs=2) as work,
            tc.tile_pool(name="scal", bufs=1) as scal,
            tc.tile_pool(name="ps_t", bufs=1, space="PSUM") as ps_t,
            tc.tile_pool(name="ps_v", bufs=1, space="PSUM") as ps_v,
            tc.tile_pool(name="ps_z", bufs=1, space="PSUM") as ps_z,
        ):
            # ---------- constants ----------
            ident = wpool.tile([128, 128], f16, tag="ident")
            masks.make_identity(nc, ident[:])
            ones1 = wpool.tile([1, 128], f16, tag="ones1")
            nc.vector.memset(ones1[:], 1.0)
            eps_t = wpool.tile([T, 1], f32, tag="eps_t")
            nc.vector.memset(eps_t[:], EPS)

            # ---------- small DMAs first (sync queue) ----------
            g_rows = []
            for m in range(NL):
                gr = work.tile([1, F], f16, tag="g_row", bufs=4, name=f"g_row{m}")
                nc.sync.dma_start(gr[:], g4_d[m:m + 1, :])
                g_rows.append(gr)
            s_rows = {}
            for i in range(1, NL):
                sr = work.tile([1, D], f16, tag="s_row", bufs=3, name=f"s_row{i}")
                nc.sync.dma_start(sr[:], s4_d[i:i + 1, :])
                s_rows[i] = sr
            su_rows = {}
            for i in range(1, NL):
                sr = work.tile([1, F], f16, tag="su_row", bufs=3, name=f"su_row{i}")
                nc.sync.dma_start(sr[:], su4_d[i:i + 1, :])
                su_rows[i] = sr
            h = work.tile([T, D], f16, tag="h", bufs=2)
            nc.sync.dma_start(h[:], h0_d[:, :])
            hT0 = work.tile([T, D], f16, tag="hT0", bufs=1)
            for k in range(6):
                nc.sync.dma_start(hT0[:, 128 * k:128 * (k + 1)],
                                  h0T_d[128 * k:128 * (k + 1), :])

            winT_sb = []
            for k in range(6):
                wt = wpool.tile([128, F], f16, tag=f"winT{k}")
                nc.sync.dma_start(wt[:], winT_d[128 * k:128 * (k + 1), :])
                winT_sb.append(wt)
            woutT_sb = []
            for c in range(12):
                wt = wpool.tile([128, D], f16, tag=f"woutT{c}")
                nc.sync.dma_start(wt[:], woutT_d[128 * c:128 * (c + 1), :])
                woutT_sb.append(wt)

            SL = [slice(512 * n, 512 * (n + 1)) for n in range(3)]

            # ---------- broadcast tiles via PE ones-trick ----------
            def bcast_f(row, tag):  # [1, F] -> [128, F]
                sb = wpool.tile([T, F], f16, tag=tag)
                for n in range(3):
                    ps = ps_v.tile([T, 512], f32, tag=f"v{n}", name=f"bc_{tag}{n}")
                    nc.tensor.matmul(ps[:, :], ones1[:, :], row[:, SL[n]],
                                     start=True, stop=True)
                    nc.scalar.copy(sb[:, SL[n]], ps[:, :])
                return sb

            def bcast_d(row, tag):  # [1, D] -> [128, D]
                ps = ps_z.tile([T, D], f32, tag="z")
                for off, nn in ((0, 512), (512, 256)):
                    nc.tensor.matmul(ps[:, off:off + nn], ones1[:, :],
                                     row[:, off:off + nn], start=True, stop=True)
                sb = wpool.tile([T, D], f16, tag=tag)
                nc.scalar.copy(sb[:], ps[:])
                return sb

            Gb = [bcast_f(g_rows[m], f"Gb{m}") for m in range(NL)]
            SU = {i: bcast_f(su_rows[i], f"SU{i}") for i in range(1, NL)}
            Sb = {i: bcast_d(s_rows[i], f"Sb{i}") for i in range(1, NL)}

            accs = {j: apool.tile([T, F], f16, tag=f"acc{j}", name=f"acc{j}")
                    for j in (1, 2, 3)}
            rs_w_p = a_p = None
            u_prev = None
            zT_sb = None

            # Warm-keeper: dummy matmuls into a spare PSUM bank keep the PE
            # HAM clock-gate at 8/8 through dependency handoffs (PE transposes
            # do not count as HAM-busy, and any >~3.4us idle re-throttles to
            # 1.2 GHz).  FIFO placement makes them run exactly in the stalls.
            dummy_ps = ps_t.tile([T, 512], f32, tag="dummy")

            def warm(n_mm):
                for _ in range(n_mm):
                    nc.tensor.matmul(dummy_ps[:, :], ones1[:, :],
                                     g_rows[0][:, 0:512], start=True,
                                     stop=True, skip_group_check=True)

            for i in range(NL):
                first, last = i == 0, i == NL - 1

                # ---- T1_n = b*u_prev + SU_i (early window ops, old scalars)
                t1s = []
                if not first:
                    for n in range(3):
                        t1 = work.tile([T, 512], f16, tag=f"t1_{n}", bufs=2,
                                       name=f"t1_{n}")
                        nc.vector.scalar_tensor_tensor(
                            out=t1[:], in0=u_prev[:, SL[n]],
                            scalar=rs_w_p[:, :], in1=SU[i][:, SL[n]],
                            op0=AL.mult, op1=AL.add)
                        t1s.append(t1)

                # ---- MM1: v_i = (h_0 or z_{i-1}) @ W_in^T ----
                v_ps = [ps_v.tile([T, 512], f32, tag=f"v{n}", name=f"v{n}_{i}")
                        for n in range(3)]
                lhsT = hT0 if first else zT_sb
                if first:
                    for k in range(6):      # k-outer: winT DMA arrival order
                        for n in range(3):
                            nc.tensor.matmul(
                                v_ps[n][:, :], lhsT[:, 128 * k:128 * (k + 1)],
                                winT_sb[k][:, SL[n]],
                                start=(k == 0), stop=(k == 5))
                        warm(2)             # stay HAM-busy during winT DMA
                else:
                    for n in range(3):      # n-outer: slices finish early
                        for k in range(6):
                            nc.tensor.matmul(
                                v_ps[n][:, :], lhsT[:, 128 * k:128 * (k + 1)],
                                winT_sb[k][:, SL[n]],
                                start=(k == 0), stop=(k == 5))
                    warm(4)                 # cover the y-combine handoff

                # ---- per-slice: u and y; then yT transposes ----
                u_cur = work.tile([T, F], f16, tag="u", bufs=3, name=f"u{i}")
                y = work.tile([T, F], f16, tag="y", bufs=1)
                yT_ps = ps_t.tile([T, F], f16, tag="t")
                for n in range(3):
                    sl = SL[n]
                    if first:
                        # u = v (copy for acc updates, ACT); y = G0*u
                        nc.scalar.copy(u_cur[:, sl], v_ps[n][:, :])
                        nc.vector.tensor_mul(y[:, sl], v_ps[n][:, :],
                                             Gb[0][:, sl])
                    else:
                        # u = a*v + T1   (critical)
                        nc.vector.scalar_tensor_tensor(
                            out=u_cur[:, sl], in0=v_ps[n][:, :],
                            scalar=a_p[:, :], in1=t1s[n][:],
                            op0=AL.mult, op1=AL.add)
                        # y = G0*u + acc_i
                        ym = work.tile([T, 512], f16, tag=f"ym_{n}", bufs=2,
                                       name=f"ym_{n}")
                        nc.vector.tensor_mul(ym[:], u_cur[:, sl], Gb[0][:, sl])
                        nc.vector.tensor_add(y[:, sl], ym[:], accs[i][:, sl])
                    for c in range(4 * n, 4 * n + 4):
                        nc.tensor.transpose(
                            yT_ps[:, 128 * c:128 * (c + 1)],
                            y[:, 128 * c:128 * (c + 1)], ident[:])

                yT_sb = work.tile([T, F], f16, tag="yT_sb", bufs=1)
                for n in range(3):
                    nc.scalar.copy(yT_sb[:, SL[n]], yT_ps[:, SL[n]])

                # ---- eager acc updates for future loops (off-path; muls on
                # DVE - gpsimd TT is ~3.8us and its SBUF-port lockout slows
                # concurrent DVE ops, so gpsimd only gets the rare adds) ----
                for j in range(i + 1, NL):
                    m = j - i
                    if first:
                        nc.vector.tensor_mul(accs[j][:], u_cur[:], Gb[m][:])
                    else:
                        tmp_a = work.tile([T, F], f16, tag="tmp_a", bufs=2)
                        nc.vector.tensor_mul(tmp_a[:], u_cur[:], Gb[m][:])
                        nc.gpsimd.tensor_add(accs[j][:], accs[j][:], tmp_a[:])

                # ---- MM2: z = y @ out_proj^T ----
                z_ps = ps_z.tile([T, D], f32, tag="z")
                for c in range(12):
                    for off, nn in ((0, 512), (512, 256)):
                        nc.tensor.matmul(
                            z_ps[:, off:off + nn],
                            yT_sb[:, 128 * c:128 * (c + 1)],
                            woutT_sb[c][:, off:off + nn],
                            start=(c == 0), stop=(c == 11))
                    if first:
                        warm(1)             # stay HAM-busy during woutT DMA
                warm(6)                     # cover the z->zT handoff

                # ---- z -> SBUF f16; zT (next loop's MM1 stationary) ----
                z_sb = work.tile([T, D], f16, tag="z_sb", bufs=2)
                nc.scalar.copy(z_sb[:, 0:384], z_ps[:, 0:384])
                nc.scalar.copy(z_sb[:, 384:768], z_ps[:, 384:768])
                if not last:
                    zT_ps = ps_t.tile([T, D], f16, tag="t")
                    for k in range(6):
                        nc.tensor.transpose(
                            zT_ps[:, 128 * k:128 * (k + 1)],
                            z_sb[:, 128 * k:128 * (k + 1)], ident[:])
                    zT_sb = work.tile([T, D], f16, tag="zT_sb", bufs=2)
                    nc.scalar.copy(zT_sb[:, 0:384], zT_ps[:, 0:384])
                    nc.scalar.copy(zT_sb[:, 384:768], zT_ps[:, 384:768])

                # ---- rmsnorm chain (direct form; deadlines are loose:
                # rs_w/a feed the NEXT loop's window ops only) ----
                ss_z = scal.tile([T, 1], f32, tag="ss_z", bufs=2)
                sq_scr = work.tile([T, D], f16, tag="scr", bufs=3)
                nc.scalar.activation(sq_scr[:], z_ps[:], AF.Square,
                                     accum_out=ss_z[:])
                sq_z = scal.tile([T, 1], f32, tag="sq_z", bufs=2)
                nc.scalar.activation(sq_z[:], ss_z[:], AF.Sqrt,
                                     bias=eps_t[:, :], scale=1.0 / D)
                rs_z = scal.tile([T, 1], f32, tag="rs_z", bufs=2)
                nc.vector.reciprocal(rs_z[:], sq_z[:])

                # w = z*rs_z + h
                last_dt = f32 if last else f16
                w = work.tile([T, D], last_dt, tag="w", bufs=2, name=f"w{i}")
                nc.vector.scalar_tensor_tensor(
                    out=w[:], in0=z_sb[:], scalar=rs_z[:, :], in1=h[:],
                    op0=AL.mult, op1=AL.add)
                ss_w = scal.tile([T, 1], f32, tag="ss_w", bufs=2)
                sq_scr2 = work.tile([T, D], f16, tag="scr", bufs=3)
                nc.scalar.activation(sq_scr2[:], w[:], AF.Square,
                                     accum_out=ss_w[:])
                sq_w = scal.tile([T, 1], f32, tag="sq_w", bufs=2)
                nc.scalar.activation(sq_w[:], ss_w[:], AF.Sqrt,
                                     bias=eps_t[:, :], scale=1.0 / D)
                rs_w = scal.tile([T, 1], f32, tag="rs_w", bufs=2)
                nc.vector.reciprocal(rs_w[:], sq_w[:])
                a_t = scal.tile([T, 1], f32, tag="a_t", bufs=2)
                nc.vector.tensor_mul(a_t[:], rs_z[:], rs_w[:])

                # ---- h update: h' = w*rs_w + Sb_{i+1} ----
                if not last:
                    h_next = work.tile([T, D], f16, tag="h", bufs=2)
                    nc.vector.scalar_tensor_tensor(
                        out=h_next[:], in0=w[:], scalar=rs_w[:, :],
                        in1=Sb[i + 1][:], op0=AL.mult, op1=AL.add)
                    h = h_next
                else:
                    x_out = work.tile([T, D], f32, tag="x_out", bufs=1)
                    nc.vector.tensor_scalar_mul(x_out[:], w[:], rs_w[:, :])
                    nc.sync.dma_start(out_d[:, :], x_out[:])

                rs_w_p, a_p = rs_w, a_t
                u_prev = u_cur

    nc.compile()
    return nc


def _host_prep(x, in_proj_base, lora_A, lora_B, A_theta, B_real, B_imag,
               C_real, C_imag, out_proj_w, step_emb):
    W_in = in_proj_base.astype(np.float64) + 2.0 * (
        lora_B.astype(np.float64) @ lora_A.astype(np.float64))
    winT = np.ascontiguousarray(W_in.T).astype(np.float16)
    woutT = np.ascontiguousarray(out_proj_w.T).astype(np.float16)

    th = A_theta.astype(np.float64)
    P = (C_real.astype(np.float64) * B_real.astype(np.float64)
         - C_imag.astype(np.float64) * B_imag.astype(np.float64))
    Q = (C_real.astype(np.float64) * B_imag.astype(np.float64)
         + C_imag.astype(np.float64) * B_real.astype(np.float64))
    g4 = np.stack([
        (P * np.cos(m * th) - Q * np.sin(m * th)).sum(-1).reshape(-1)
        for m in range(NL)
    ]).astype(np.float16)                                   # [4, 1536]
    s4 = np.ascontiguousarray(step_emb).astype(np.float16)  # [4, 768]
    su4 = (step_emb.astype(np.float64) @ W_in.T).astype(np.float16)  # [4,1536]

    h0 = (x[0].astype(np.float64) + step_emb[0].astype(np.float64)
          ).astype(np.float16)                              # [1024, 768]
    h0T = np.ascontiguousarray(h0.T)                        # [768, 1024]
    ssh0 = (h0.astype(np.float32) ** 2).sum(-1, keepdims=True)  # [1024, 1]
    return winT, woutT, g4, s4, su4, h0, h0T, ssh0


def kernel(x, in_proj_base, lora_A, lora_B, A_theta, B_real, B_imag,
           C_real, C_imag, out_proj_w, mixer_norm_w, loop_norm_w, step_emb,
           _trace=False):
    winT, woutT, g4, s4, su4, h0, h0T, ssh0 = _host_prep(
        np.asarray(x, dtype=np.float32), np.asarray(in_proj_base),
        np.asarray(lora_A), np.asarray(lora_B), np.asarray(A_theta),
        np.asarray(B_real), np.asarray(B_imag), np.asarray(C_real),
        np.asarray(C_imag), np.asarray(out_proj_w), np.asarray(step_emb))
    # mixer_norm_w / loop_norm_w are ones per the problem spec; rmsnorm weight
    # multiplies are identity and omitted on device.

    if "nc" not in _CACHE:
        _CACHE["nc"] = build_nc()
    nc = _CACHE["nc"]

    shared = {"winT": winT, "woutT": woutT, "g4": g4, "s4": s4, "su4": su4}
    in_maps = [
        {**shared,
         "h0": np.ascontiguousarray(h0[T * c:T * (c + 1), :]),
         "h0T": np.ascontiguousarray(h0T[:, T * c:T * (c + 1)])}
        for c in range(NCORES)
    ]
    res = run_bass_kernel_spmd(nc, in_maps, list(range(NCORES)), trace=_trace)
    out = np.concatenate(
        [np.asarray(res.results[c]["x_out"]) for c in range(NCORES)], axis=0)
    if _trace:
        _CACHE["last_result"] = res
    return out[None, :, :].astype(np.float32)


# revision 24
# speedup vs baseline: 1.4714x; 1.4117x over previous
"""Trainium2 Bass kernel for RecursiveMamba130M (fp16, v-recursion pipeline).

Math: the complex SSM state collapses to a depthwise convolution over
reasoning loops:
  y_i[t, f] = sum_{k<=i} G_{i-k}[f] * u_k[t, f],   u_k = h_k @ W_in^T
  G_m[f] = sum_s (CrBr - CiBi) cos(m th) - (CrBi + CiBr) sin(m th)

Key restructure: h_{i+1} = a*z_i + b*h_i + step_{i+1} with per-token
scalars a = rs_z*rs_w, b = rs_w.  Right-multiplying by W_in^T:
  u_{i+1} = a*(z_i @ W_in^T) + b*u_i + su_{i+1},   su = step @ W_in^T
so MM1 of loop i+1 becomes v = z_i @ W_in^T, which depends only on z_i
(NOT on the rmsnorm scalars).  The PE therefore flows
MM2 -> transpose(z) -> MM1' -> transpose(y) -> MM2 with no norm stall;
the rmsnorm chain, u/h recovery and acc updates overlap under MM1'/MM2
(their deadlines are one loop away, so queue latency is hidden).

Sharding: data-parallel over sequence: 128 tokens per core, no
collectives; weights replicated.  fp16 data / fp32 PSUM + scalars.
h0/h0T and su = step_emb @ W_in^T are precomputed on host so loop 0
starts on the first winT DMA chunk (k-outer order matches arrival).
"""

import numpy as np

import concourse.bass as bass
import concourse.tile as tile
from concourse.bacc import Bacc
from concourse import masks, mybir
from concourse.bass_utils import run_bass_kernel_spmd

T = 128          # tokens per core
D = 768          # d_model
F = 1536         # 2 * d_model
NL = 4           # reasoning loops
NCORES = 8
EPS = 1e-6

f16 = mybir.dt.float16
f32 = mybir.dt.float32
AL = mybir.AluOpType
AF = mybir.ActivationFunctionType

_CACHE = {}


def build_nc():
    nc = Bacc()
    h0_d = nc.dram_tensor("h0", [T, D], f16, kind="ExternalInput")
    h0T_d = nc.dram_tensor("h0T", [D, T], f16, kind="ExternalInput")
    winT_d = nc.dram_tensor("winT", [D, F], f16, kind="ExternalInput")
    woutT_d = nc.dram_tensor("woutT", [F, D], f16, kind="ExternalInput")
    g4_d = nc.dram_tensor("g4", [NL, F], f16, kind="ExternalInput")
    s4_d = nc.dram_tensor("s4", [NL, D], f16, kind="ExternalInput")
    su4_d = nc.dram_tensor("su4", [NL, F], f16, kind="ExternalInput")
    out_d = nc.dram_tensor("x_out", [T, D], f32, kind="ExternalOutput")

    with tile.TileContext(nc) as tc:
        with (
            tc.tile_pool(name="wpool", bufs=1) as wpool,
            tc.tile_pool(name="apool", bufs=1) as apool,
            tc.tile_pool(name="work", bufs=2) as work,
            tc.tile_pool(name="scal", bufs=1) as scal,
            tc.tile_pool(name="ps_t", bufs=1, space="PSUM") as ps_t,
            tc.tile_pool(name="ps_v", bufs=1, space="PSUM") as ps_v,
            tc.tile_pool(name="ps_z", bufs=1, space="PSUM") as ps_z,
        ):
            # ---------- constants ----------
            ident = wpool.tile([128, 128], f16, tag="ident")
            masks.make_identity(nc, ident[:])
            ones1 = wpool.tile([1, 128], f16, tag="ones1")
            nc.vector.memset(ones1[:], 1.0)
            eps_t = wpool.tile([T, 1], f32, tag="eps_t")
            nc.vector.memset(eps_t[:], EPS)

            # ---------- small DMAs first (sync queue) ----------
            g_rows = []
            for m in range(NL):
                gr = work.tile([1, F], f16, tag="g_row", bufs=4, name=f"g_row{m}")
                nc.sync.dma_start(gr[:], g4_d[m:m + 1, :])
                g_rows.append(gr)
            s_rows = {}
            for i in range(1, NL):
                sr = work.tile([1, D], f16, tag="s_row", bufs=3, name=f"s_row{i}")
                nc.sync.dma_start(sr[:], s4_d[i:i + 1, :])
                s_rows[i] = sr
            su_rows = {}
            for i in range(1, NL):
                sr = work.tile([1, F], f16, tag="su_row", bufs=3, name=f"su_row{i}")
                nc.sync.dma_start(sr[:], su4_d[i:i + 1, :])
                su_rows[i] = sr
            h = work.tile([T, D], f16, tag="h", bufs=2)
            nc.sync.dma_start(h[:], h0_d[:, :])
            hT0 = work.tile([T, D], f16, tag="hT0", bufs=1)
            for k in range(6):
                nc.sync.dma_start(hT0[:, 128 * k:128 * (k + 1)],
                                  h0T_d[128 * k:128 * (k + 1), :])

            winT_sb = []
            for k in range(6):
                wt = wpool.tile([128, F], f16, tag=f"winT{k}")
                nc.sync.dma_start(wt[:], winT_d[128 * k:128 * (k + 1), :])
                winT_sb.append(wt)
            woutT_sb = []
            for c in range(12):
                wt = wpool.tile([128, D], f16, tag=f"woutT{c}")
                nc.sync.dma_start(wt[:], woutT_d[128 * c:128 * (c + 1), :])
                woutT_sb.append(wt)

            SL = [slice(512 * n, 512 * (n + 1)) for n in range(3)]

            # ---------- broadcast tiles via PE ones-trick ----------
            def bcast_f(row, tag):  # [1, F] -> [128, F]
                sb = wpool.tile([T, F], f16, tag=tag)
                for n in range(3):
                    ps = ps_v.tile([T, 512], f32, tag=f"v{n}", name=f"bc_{tag}{n}")
                    nc.tensor.matmul(ps[:, :], ones1[:, :], row[:, SL[n]],
                                     start=True, stop=True)
                    nc.scalar.copy(sb[:, SL[n]], ps[:, :])
                return sb

            def bcast_d(row, tag):  # [1, D] -> [128, D]
                ps = ps_z.tile([T, D], f32, tag="z")
                for off, nn in ((0, 512), (512, 256)):
                    nc.tensor.matmul(ps[:, off:off + nn], ones1[:, :],
                                     row[:, off:off + nn], start=True, stop=True)
                sb = wpool.tile([T, D], f16, tag=tag)
                nc.scalar.copy(sb[:], ps[:])
                return sb

            Gb = [bcast_f(g_rows[m], f"Gb{m}") for m in range(NL)]
            SU = {i: bcast_f(su_rows[i], f"SU{i}") for i in range(1, NL)}
            Sb = {i: bcast_d(s_rows[i], f"Sb{i}") for i in range(1, NL)}

            accs = {j: apool.tile([T, F], f16, tag=f"acc{j}", name=f"acc{j}")
                    for j in (1, 2, 3)}
            rs_w_p = a_p = None
            u_prev = None
            zT_sb = None

            for i in range(NL):
                first, last = i == 0, i == NL - 1

                # ---- T1_n = b*u_prev + SU_i (early window ops, old scalars)
                t1s = []
                if not first:
                    for n in range(3):
                        t1 = work.tile([T, 512], f16, tag=f"t1_{n}", bufs=2,
                                       name=f"t1_{n}")
                        nc.vector.scalar_tensor_tensor(
                            out=t1[:], in0=u_prev[:, SL[n]],
                            scalar=rs_w_p[:, :], in1=SU[i][:, SL[n]],
                            op0=AL.mult, op1=AL.add)
                        t1s.append(t1)

                # ---- MM1: v_i = (h_0 or z_{i-1}) @ W_in^T ----
                v_ps = [ps_v.tile([T, 512], f32, tag=f"v{n}", name=f"v{n}_{i}")
                        for n in range(3)]
                lhsT = hT0 if first else zT_sb
                if first:
                    for k in range(6):      # k-outer: winT DMA arrival order
                        for n in range(3):
                            nc.tensor.matmul(
                                v_ps[n][:, :], lhsT[:, 128 * k:128 * (k + 1)],
                                winT_sb[k][:, SL[n]],
                                start=(k == 0), stop=(k == 5))
                else:
                    for n in range(3):      # n-outer: slices finish early
                        for k in range(6):
                            nc.tensor.matmul(
                                v_ps[n][:, :], lhsT[:, 128 * k:128 * (k + 1)],
                                winT_sb[k][:, SL[n]],
                                start=(k == 0), stop=(k == 5))

                # ---- per-slice: u and y; then yT transposes ----
                u_cur = work.tile([T, F], f16, tag="u", bufs=2, name=f"u{i}")
                y = work.tile([T, F], f16, tag="y", bufs=1)
                yT_ps = ps_t.tile([T, F], f16, tag="t")
                for n in range(3):
                    sl = SL[n]
                    if first:
                        # u = v (copy for acc updates, ACT); y = G0*u
                        nc.scalar.copy(u_cur[:, sl], v_ps[n][:, :])
                        nc.vector.tensor_mul(y[:, sl], v_ps[n][:, :],
                                             Gb[0][:, sl])
                    else:
                        # u = a*v + T1   (critical)
                        nc.vector.scalar_tensor_tensor(
                            out=u_cur[:, sl], in0=v_ps[n][:, :],
                            scalar=a_p[:, :], in1=t1s[n][:],
                            op0=AL.mult, op1=AL.add)
                        # y = G0*u + acc_i
                        ym = work.tile([T, 512], f16, tag=f"ym_{n}", bufs=2,
                                       name=f"ym_{n}")
                        nc.vector.tensor_mul(ym[:], u_cur[:, sl], Gb[0][:, sl])
                        nc.vector.tensor_add(y[:, sl], ym[:], accs[i][:, sl])
                    for c in range(4 * n, 4 * n + 4):
                        nc.tensor.transpose(
                            yT_ps[:, 128 * c:128 * (c + 1)],
                            y[:, 128 * c:128 * (c + 1)], ident[:])

                yT_sb = work.tile([T, F], f16, tag="yT_sb", bufs=1)
                for n in range(3):
                    nc.scalar.copy(yT_sb[:, SL[n]], yT_ps[:, SL[n]])

                # ---- eager acc updates for future loops (off-path; muls on
                # DVE - gpsimd TT is ~3.8us and its SBUF-port lockout slows
                # concurrent DVE ops, so gpsimd only gets the rare adds) ----
                for j in range(i + 1, NL):
                    m = j - i
                    if first:
                        nc.vector.tensor_mul(accs[j][:], u_cur[:], Gb[m][:])
                    else:
                        tmp_a = work.tile([T, F], f16, tag="tmp_a", bufs=2)
                        nc.vector.tensor_mul(tmp_a[:], u_cur[:], Gb[m][:])
                        nc.gpsimd.tensor_add(accs[j][:], accs[j][:], tmp_a[:])

                # ---- MM2: z = y @ out_proj^T ----
                z_ps = ps_z.tile([T, D], f32, tag="z")
                for c in range(12):
                    for off, nn in ((0, 512), (512, 256)):
                        nc.tensor.matmul(
                            z_ps[:, off:off + nn],
                            yT_sb[:, 128 * c:128 * (c + 1)],
                            woutT_sb[c][:, off:off + nn],
                            start=(c == 0), stop=(c == 11))

                # ---- z -> SBUF f16; zT (next loop's MM1 stationary) ----
                z_sb = work.tile([T, D], f16, tag="z_sb", bufs=2)
                nc.scalar.copy(z_sb[:, 0:384], z_ps[:, 0:384])
                nc.scalar.copy(z_sb[:, 384:768], z_ps[:, 384:768])
                if not last:
                    zT_ps = ps_t.tile([T, D], f16, tag="t")
                    for k in range(6):
                        nc.tensor.transpose(
                            zT_ps[:, 128 * k:128 * (k + 1)],
                            z_sb[:, 128 * k:128 * (k + 1)], ident[:])
                    zT_sb = work.tile([T, D], f16, tag="zT_sb", bufs=2)
                    nc.scalar.copy(zT_sb[:, 0:384], zT_ps[:, 0:384])
                    nc.scalar.copy(zT_sb[:, 384:768], zT_ps[:, 384:768])

                # ---- rmsnorm chain (direct form; deadlines are loose:
                # rs_w/a feed the NEXT loop's window ops only) ----
                ss_z = scal.tile([T, 1], f32, tag="ss_z", bufs=2)
                sq_scr = work.tile([T, D], f16, tag="scr", bufs=3)
                nc.scalar.activation(sq_scr[:], z_ps[:], AF.Square,
                                     accum_out=ss_z[:])
                sq_z = scal.tile([T, 1], f32, tag="sq_z", bufs=2)
                nc.scalar.activation(sq_z[:], ss_z[:], AF.Sqrt,
                                     bias=eps_t[:, :], scale=1.0 / D)
                rs_z = scal.tile([T, 1], f32, tag="rs_z", bufs=2)
                nc.vector.reciprocal(rs_z[:], sq_z[:])

                # w = z*rs_z + h
                last_dt = f32 if last else f16
                w = work.tile([T, D], last_dt, tag="w", bufs=2, name=f"w{i}")
                nc.vector.scalar_tensor_tensor(
                    out=w[:], in0=z_sb[:], scalar=rs_z[:, :], in1=h[:],
                    op0=AL.mult, op1=AL.add)
                ss_w = scal.tile([T, 1], f32, tag="ss_w", bufs=2)
                sq_scr2 = work.tile([T, D], f16, tag="scr", bufs=3)
                nc.scalar.activation(sq_scr2[:], w[:], AF.Square,
                                     accum_out=ss_w[:])
                sq_w = scal.tile([T, 1], f32, tag="sq_w", bufs=2)
                nc.scalar.activation(sq_w[:], ss_w[:], AF.Sqrt,
                                     bias=eps_t[:, :], scale=1.0 / D)
                rs_w = scal.tile([T, 1], f32, tag="rs_w", bufs=2)
                nc.vector.reciprocal(rs_w[:], sq_w[:])
                a_t = scal.tile([T, 1], f32, tag="a_t", bufs=2)
                nc.vector.tensor_mul(a_t[:], rs_z[:], rs_w[:])

                # ---- h update: h' = w*rs_w + Sb_{i+1} ----
                if not last:
                    h_next = work.tile([T, D], f16, tag="h", bufs=2)
                    nc.vector.scalar_tensor_tensor(
                        out=h_next[:], in0=w[:], scalar=rs_w[:, :],
                        in1=Sb[i + 1][:], op0=AL.mult, op1=AL.add)
                    h = h_next
                else:
                    x_out = work.tile([T, D], f32, tag="x_out", bufs=1)
                    nc.vector.tensor_scalar_mul(x_out[:], w[:], rs_w[:, :])
                    nc.sync.dma_start(out_d[:, :], x_out[:])

                rs_w_p, a_p = rs_w, a_t
                u_prev = u_cur

    nc.compile()
    return nc


def _host_prep(x, in_proj_base, lora_A, lora_B, A_theta, B_real, B_imag,
               C_real, C_imag, out_proj_w, step_emb):
    W_in = in_proj_base.astype(np.float64) + 2.0 * (
        lora_B.astype(np.float64) @ lora_A.astype(np.float64))
    winT = np.ascontiguousarray(W_in.T).astype(np.float16)
    woutT = np.ascontiguousarray(out_proj_w.T).astype(np.float16)

    th = A_theta.astype(np.float64)
    P = (C_real.astype(np.float64) * B_real.astype(np.float64)
         - C_imag.astype(np.float64) * B_imag.astype(np.float64))
    Q = (C_real.astype(np.float64) * B_imag.astype(np.float64)
         + C_imag.astype(np.float64) * B_real.astype(np.float64))
    g4 = np.stack([
        (P * np.cos(m * th) - Q * np.sin(m * th)).sum(-1).reshape(-1)
        for m in range(NL)
    ]).astype(np.float16)                                   # [4, 1536]
    s4 = np.ascontiguousarray(step_emb).astype(np.float16)  # [4, 768]
    su4 = (step_emb.astype(np.float64) @ W_in.T).astype(np.float16)  # [4,1536]

    h0 = (x[0].astype(np.float64) + step_emb[0].astype(np.float64)
          ).astype(np.float16)                              # [1024, 768]
    h0T = np.ascontiguousarray(h0.T)                        # [768, 1024]
    ssh0 = (h0.astype(np.float32) ** 2).sum(-1, keepdims=True)  # [1024, 1]
    return winT, woutT, g4, s4, su4, h0, h0T, ssh0


def kernel(x, in_proj_base, lora_A, lora_B, A_theta, B_real, B_imag,
           C_real, C_imag, out_proj_w, mixer_norm_w, loop_norm_w, step_emb,
           _trace=False):
    winT, woutT, g4, s4, su4, h0, h0T, ssh0 = _host_prep(
        np.asarray(x, dtype=np.float32), np.asarray(in_proj_base),
        np.asarray(lora_A), np.asarray(lora_B), np.asarray(A_theta),
        np.asarray(B_real), np.asarray(B_imag), np.asarray(C_real),
        np.asarray(C_imag), np.asarray(out_proj_w), np.asarray(step_emb))
    # mixer_norm_w / loop_norm_w are ones per the problem spec; rmsnorm weight
    # multiplies are identity and omitted on device.

    if "nc" not in _CACHE:
        _CACHE["nc"] = build_nc()
    nc = _CACHE["nc"]

    shared = {"winT": winT, "woutT": woutT, "g4": g4, "s4": s4, "su4": su4}
    in_maps = [
        {**shared,
         "h0": np.ascontiguousarray(h0[T * c:T * (c + 1), :]),
         "h0T": np.ascontiguousarray(h0T[:, T * c:T * (c + 1)])}
        for c in range(NCORES)
    ]
    res = run_bass_kernel_spmd(nc, in_maps, list(range(NCORES)), trace=_trace)
    out = np.concatenate(
        [np.asarray(res.results[c]["x_out"]) for c in range(NCORES)], axis=0)
    if _trace:
        _CACHE["last_result"] = res
    return out[None, :, :].astype(np.float32)


# revision 25
# speedup vs baseline: 1.4966x; 1.0171x over previous
"""Trainium2 Bass kernel for RecursiveMamba130M (fp16, v-recursion pipeline).

Math: the complex SSM state collapses to a depthwise convolution over
reasoning loops:
  y_i[t, f] = sum_{k<=i} G_{i-k}[f] * u_k[t, f],   u_k = h_k @ W_in^T
  G_m[f] = sum_s (CrBr - CiBi) cos(m th) - (CrBi + CiBr) sin(m th)

Key restructure: h_{i+1} = a*z_i + b*h_i + step_{i+1} with per-token
scalars a = rs_z*rs_w, b = rs_w.  Right-multiplying by W_in^T:
  u_{i+1} = a*(z_i @ W_in^T) + b*u_i + su_{i+1},   su = step @ W_in^T
so MM1 of loop i+1 becomes v = z_i @ W_in^T, which depends only on z_i
(NOT on the rmsnorm scalars).  The PE therefore flows
MM2 -> transpose(z) -> MM1' -> transpose(y) -> MM2 with no norm stall;
the rmsnorm chain, u/h recovery and acc updates overlap under MM1'/MM2
(their deadlines are one loop away, so queue latency is hidden).

Sharding: data-parallel over sequence: 128 tokens per core, no
collectives; weights replicated.  fp16 data / fp32 PSUM + scalars.
h0/h0T and su = step_emb @ W_in^T are precomputed on host so loop 0
starts on the first winT DMA chunk (k-outer order matches arrival).
"""

import numpy as np

import concourse.bass as bass
import concourse.tile as tile
from concourse.bacc import Bacc
from concourse import masks, mybir
from concourse.bass_utils import run_bass_kernel_spmd

T = 128          # tokens per core
D = 768          # d_model
F = 1536         # 2 * d_model
NL = 4           # reasoning loops
NCORES = 8
EPS = 1e-6

f16 = mybir.dt.float16
f32 = mybir.dt.float32
AL = mybir.AluOpType
AF = mybir.ActivationFunctionType

_CACHE = {}


def build_nc():
    nc = Bacc()
    h0_d = nc.dram_tensor("h0", [T, D], f16, kind="ExternalInput")
    h0T_d = nc.dram_tensor("h0T", [D, T], f16, kind="ExternalInput")
    winT_d = nc.dram_tensor("winT", [D, F], f16, kind="ExternalInput")
    woutT_d = nc.dram_tensor("woutT", [F, D], f16, kind="ExternalInput")
    g4_d = nc.dram_tensor("g4", [NL, F], f16, kind="ExternalInput")
    s4_d = nc.dram_tensor("s4", [NL, D], f16, kind="ExternalInput")
    su4_d = nc.dram_tensor("su4", [NL, F], f16, kind="ExternalInput")
    out_d = nc.dram_tensor("x_out", [T, D], f32, kind="ExternalOutput")

    with tile.TileContext(nc) as tc:
        with (
            tc.tile_pool(name="wpool", bufs=1) as wpool,
            tc.tile_pool(name="apool", bufs=1) as apool,
            tc.tile_pool(name="work", bufs=2) as work,
            tc.tile_pool(name="scal", bufs=1) as scal,
            tc.tile_pool(name="ps_t", bufs=1, space="PSUM") as ps_t,
            tc.tile_pool(name="ps_v", bufs=1, space="PSUM") as ps_v,
            tc.tile_pool(name="ps_z", bufs=1, space="PSUM") as ps_z,
        ):
            # ---------- constants ----------
            ident = wpool.tile([128, 128], f16, tag="ident")
            masks.make_identity(nc, ident[:])
            ones1 = wpool.tile([1, 128], f16, tag="ones1")
            nc.vector.memset(ones1[:], 1.0)
            eps_t = wpool.tile([T, 1], f32, tag="eps_t")
            nc.vector.memset(eps_t[:], EPS)

            # ---------- small DMAs first (sync queue) ----------
            g_rows = []
            for m in range(NL):
                gr = work.tile([1, F], f16, tag="g_row", bufs=4, name=f"g_row{m}")
                nc.sync.dma_start(gr[:], g4_d[m:m + 1, :])
                g_rows.append(gr)
            s_rows = {}
            for i in range(1, NL):
                sr = work.tile([1, D], f16, tag="s_row", bufs=3, name=f"s_row{i}")
                nc.sync.dma_start(sr[:], s4_d[i:i + 1, :])
                s_rows[i] = sr
            su_rows = {}
            for i in range(1, NL):
                sr = work.tile([1, F], f16, tag="su_row", bufs=3, name=f"su_row{i}")
                nc.sync.dma_start(sr[:], su4_d[i:i + 1, :])
                su_rows[i] = sr
            h = work.tile([T, D], f16, tag="h", bufs=2)
            nc.sync.dma_start(h[:], h0_d[:, :])
            hT0 = work.tile([T, D], f16, tag="hT0", bufs=1)
            for k in range(6):
                nc.sync.dma_start(hT0[:, 128 * k:128 * (k + 1)],
                                  h0T_d[128 * k:128 * (k + 1), :])

            winT_sb = []
            for k in range(6):
                wt = wpool.tile([128, F], f16, tag=f"winT{k}")
                nc.sync.dma_start(wt[:], winT_d[128 * k:128 * (k + 1), :])
                winT_sb.append(wt)
            woutT_sb = []
            for c in range(12):
                wt = wpool.tile([128, D], f16, tag=f"woutT{c}")
                nc.sync.dma_start(wt[:], woutT_d[128 * c:128 * (c + 1), :])
                woutT_sb.append(wt)

            SL = [slice(512 * n, 512 * (n + 1)) for n in range(3)]

            # ---------- broadcast tiles via PE ones-trick ----------
            def bcast_f(row, tag):  # [1, F] -> [128, F]
                sb = wpool.tile([T, F], f16, tag=tag)
                for n in range(3):
                    ps = ps_v.tile([T, 512], f32, tag=f"v{n}", name=f"bc_{tag}{n}")
                    nc.tensor.matmul(ps[:, :], ones1[:, :], row[:, SL[n]],
                                     start=True, stop=True)
                    nc.scalar.copy(sb[:, SL[n]], ps[:, :])
                return sb

            def bcast_d(row, tag):  # [1, D] -> [128, D]
                ps = ps_z.tile([T, D], f32, tag="z")
                for off, nn in ((0, 512), (512, 256)):
                    nc.tensor.matmul(ps[:, off:off + nn], ones1[:, :],
                                     row[:, off:off + nn], start=True, stop=True)
                sb = wpool.tile([T, D], f16, tag=tag)
                nc.scalar.copy(sb[:], ps[:])
                return sb

            Gb = [bcast_f(g_rows[m], f"Gb{m}") for m in range(NL)]
            SU = {i: bcast_f(su_rows[i], f"SU{i}") for i in range(1, NL)}
            Sb = {i: bcast_d(s_rows[i], f"Sb{i}") for i in range(1, NL)}

            accs = {j: apool.tile([T, F], f16, tag=f"acc{j}", name=f"acc{j}")
                    for j in (1, 2, 3)}
            rs_w_p = a_p = None
            u_prev = None
            zT_sb = None
            pend_h = None

            for i in range(NL):
                first, last = i == 0, i == NL - 1

                # ---- T1_n = b*u_prev + SU_i (early window ops, old scalars)
                t1s = []
                if not first:
                    for n in range(3):
                        t1 = work.tile([T, 512], f16, tag=f"t1_{n}", bufs=2,
                                       name=f"t1_{n}")
                        nc.vector.scalar_tensor_tensor(
                            out=t1[:], in0=u_prev[:, SL[n]],
                            scalar=rs_w_p[:, :], in1=SU[i][:, SL[n]],
                            op0=AL.mult, op1=AL.add)
                        t1s.append(t1)

                # ---- MM1: v_i = (h_0 or z_{i-1}) @ W_in^T ----
                v_ps = [ps_v.tile([T, 512], f32, tag=f"v{n}", name=f"v{n}_{i}")
                        for n in range(3)]
                lhsT = hT0 if first else zT_sb
                if first:
                    for k in range(6):      # k-outer: winT DMA arrival order
                        for n in range(3):
                            nc.tensor.matmul(
                                v_ps[n][:, :], lhsT[:, 128 * k:128 * (k + 1)],
                                winT_sb[k][:, SL[n]],
                                start=(k == 0), stop=(k == 5))
                else:
                    for n in range(3):      # n-outer: slices finish early
                        for k in range(6):
                            nc.tensor.matmul(
                                v_ps[n][:, :], lhsT[:, 128 * k:128 * (k + 1)],
                                winT_sb[k][:, SL[n]],
                                start=(k == 0), stop=(k == 5))

                # ---- per-slice: u and y; then yT transposes ----
                u_cur = work.tile([T, F], f16, tag="u", bufs=2, name=f"u{i}")
                y = work.tile([T, F], f16, tag="y", bufs=1)
                yT_ps = ps_t.tile([T, F], f16, tag="t")
                for n in range(3):
                    sl = SL[n]
                    if first:
                        # u = v (copy for acc updates, ACT); y = G0*u
                        nc.scalar.copy(u_cur[:, sl], v_ps[n][:, :])
                        nc.vector.tensor_mul(y[:, sl], v_ps[n][:, :],
                                             Gb[0][:, sl])
                    else:
                        # u = a*v + T1   (critical)
                        nc.vector.scalar_tensor_tensor(
                            out=u_cur[:, sl], in0=v_ps[n][:, :],
                            scalar=a_p[:, :], in1=t1s[n][:],
                            op0=AL.mult, op1=AL.add)
                        # y = G0*u + acc_i
                        ym = work.tile([T, 512], f16, tag=f"ym_{n}", bufs=2,
                                       name=f"ym_{n}")
                        nc.vector.tensor_mul(ym[:], u_cur[:, sl], Gb[0][:, sl])
                        nc.vector.tensor_add(y[:, sl], ym[:], accs[i][:, sl])
                    for c in range(4 * n, 4 * n + 4):
                        nc.tensor.transpose(
                            yT_ps[:, 128 * c:128 * (c + 1)],
                            y[:, 128 * c:128 * (c + 1)], ident[:])

                yT_sb = work.tile([T, F], f16, tag="yT_sb", bufs=1)
                for n in range(3):
                    nc.scalar.copy(yT_sb[:, SL[n]], yT_ps[:, SL[n]])

                # deferred h update from the previous loop (loose deadline:
                # h_i is first read by w_i in this loop's norm section)
                if pend_h is not None:
                    w_p, rs_w_pp, sb_p = pend_h
                    h_next = work.tile([T, D], f16, tag="h", bufs=2)
                    nc.vector.scalar_tensor_tensor(
                        out=h_next[:], in0=w_p[:], scalar=rs_w_pp[:, :],
                        in1=sb_p[:], op0=AL.mult, op1=AL.add)
                    h = h_next
                    pend_h = None

                # ---- eager acc updates for future loops (off-path; muls on
                # DVE - gpsimd TT is ~3.8us and its SBUF-port lockout slows
                # concurrent DVE ops, so gpsimd only gets the rare adds) ----
                for j in range(i + 1, NL):
                    m = j - i
                    if first:
                        nc.vector.tensor_mul(accs[j][:], u_cur[:], Gb[m][:])
                    else:
                        tmp_a = work.tile([T, F], f16, tag="tmp_a", bufs=2)
                        nc.vector.tensor_mul(tmp_a[:], u_cur[:], Gb[m][:])
                        nc.gpsimd.tensor_add(accs[j][:], accs[j][:], tmp_a[:])

                # ---- MM2: z = y @ out_proj^T ----
                z_ps = ps_z.tile([T, D], f32, tag="z")
                for c in range(12):
                    for off, nn in ((0, 512), (512, 256)):
                        nc.tensor.matmul(
                            z_ps[:, off:off + nn],
                            yT_sb[:, 128 * c:128 * (c + 1)],
                            woutT_sb[c][:, off:off + nn],
                            start=(c == 0), stop=(c == 11))

                # ---- z -> SBUF f16; zT (next loop's MM1 stationary) ----
                z_sb = work.tile([T, D], f16, tag="z_sb", bufs=2)
                nc.scalar.copy(z_sb[:, 0:384], z_ps[:, 0:384])
                nc.scalar.copy(z_sb[:, 384:768], z_ps[:, 384:768])
                if not last:
                    zT_ps = ps_t.tile([T, D], f16, tag="t")
                    for k in range(6):
                        nc.tensor.transpose(
                            zT_ps[:, 128 * k:128 * (k + 1)],
                            z_sb[:, 128 * k:128 * (k + 1)], ident[:])
                    zT_sb = work.tile([T, D], f16, tag="zT_sb", bufs=2)
                    nc.scalar.copy(zT_sb[:, 0:384], zT_ps[:, 0:384])
                    nc.scalar.copy(zT_sb[:, 384:768], zT_ps[:, 384:768])

                # ---- rmsnorm chain (direct form; deadlines are loose:
                # rs_w/a feed the NEXT loop's window ops only) ----
                ss_z = scal.tile([T, 1], f32, tag="ss_z", bufs=2)
                sq_scr = work.tile([T, D], f16, tag="scr", bufs=3)
                nc.scalar.activation(sq_scr[:], z_ps[:], AF.Square,
                                     accum_out=ss_z[:])
                sq_z = scal.tile([T, 1], f32, tag="sq_z", bufs=2)
                nc.scalar.activation(sq_z[:], ss_z[:], AF.Sqrt,
                                     bias=eps_t[:, :], scale=1.0 / D)
                rs_z = scal.tile([T, 1], f32, tag="rs_z", bufs=2)
                nc.vector.reciprocal(rs_z[:], sq_z[:])

                # w = z*rs_z + h
                last_dt = f32 if last else f16
                w = work.tile([T, D], last_dt, tag="w", bufs=2, name=f"w{i}")
                nc.vector.scalar_tensor_tensor(
                    out=w[:], in0=z_sb[:], scalar=rs_z[:, :], in1=h[:],
                    op0=AL.mult, op1=AL.add)
                ss_w = scal.tile([T, 1], f32, tag="ss_w", bufs=2)
                sq_scr2 = work.tile([T, D], f16, tag="scr", bufs=3)
                nc.scalar.activation(sq_scr2[:], w[:], AF.Square,
                                     accum_out=ss_w[:])
                sq_w = scal.tile([T, 1], f32, tag="sq_w", bufs=2)
                nc.scalar.activation(sq_w[:], ss_w[:], AF.Sqrt,
                                     bias=eps_t[:, :], scale=1.0 / D)
                rs_w = scal.tile([T, 1], f32, tag="rs_w", bufs=2)
                nc.vector.reciprocal(rs_w[:], sq_w[:])
                a_t = scal.tile([T, 1], f32, tag="a_t", bufs=2)
                nc.vector.tensor_mul(a_t[:], rs_z[:], rs_w[:])

                # ---- h update demoted: runs early in the NEXT loop so it
                # never delays the critical u-chain in the DVE queue ----
                if not last:
                    pend_h = (w, rs_w, Sb[i + 1])
                else:
                    x_out = work.tile([T, D], f32, tag="x_out", bufs=1)
                    nc.vector.tensor_scalar_mul(x_out[:], w[:], rs_w[:, :])
                    nc.sync.dma_start(out_d[:, :], x_out[:])

                rs_w_p, a_p = rs_w, a_t
                u_prev = u_cur

    nc.compile()
    return nc


def _host_prep(x, in_proj_base, lora_A, lora_B, A_theta, B_real, B_imag,
               C_real, C_imag, out_proj_w, step_emb):
    W_in = in_proj_base.astype(np.float64) + 2.0 * (
        lora_B.astype(np.float64) @ lora_A.astype(np.float64))
    winT = np.ascontiguousarray(W_in.T).astype(np.float16)
    woutT = np.ascontiguousarray(out_proj_w.T).astype(np.float16)

    th = A_theta.astype(np.float64)
    P = (C_real.astype(np.float64) * B_real.astype(np.float64)
         - C_imag.astype(np.float64) * B_imag.astype(np.float64))
    Q = (C_real.astype(np.float64) * B_imag.astype(np.float64)
         + C_imag.astype(np.float64) * B_real.astype(np.float64))
    g4 = np.stack([
        (P * np.cos(m * th) - Q * np.sin(m * th)).sum(-1).reshape(-1)
        for m in range(NL)
    ]).astype(np.float16)                                   # [4, 1536]
    s4 = np.ascontiguousarray(step_emb).astype(np.float16)  # [4, 768]
    su4 = (step_emb.astype(np.float64) @ W_in.T).astype(np.float16)  # [4,1536]

    h0 = (x[0].astype(np.float64) + step_emb[0].astype(np.float64)
          ).astype(np.float16)                              # [1024, 768]
    h0T = np.ascontiguousarray(h0.T)                        # [768, 1024]
    ssh0 = (h0.astype(np.float32) ** 2).sum(-1, keepdims=True)  # [1024, 1]
    return winT, woutT, g4, s4, su4, h0, h0T, ssh0


def kernel(x, in_proj_base, lora_A, lora_B, A_theta, B_real, B_imag,
           C_real, C_imag, out_proj_w, mixer_norm_w, loop_norm_w, step_emb,
           _trace=False):
    winT, woutT, g4, s4, su4, h0, h0T, ssh0 = _host_prep(
        np.asarray(x, dtype=np.float32), np.asarray(in_proj_base),
        np.asarray(lora_A), np.asarray(lora_B), np.asarray(A_theta),
        np.asarray(B_real), np.asarray(B_imag), np.asarray(C_real),
        np.asarray(C_imag), np.asarray(out_proj_w), np.asarray(step_emb))
    # mixer_norm_w / loop_norm_w are ones per the problem spec; rmsnorm weight
    # multiplies are identity and omitted on device.

    if "nc" not in _CACHE:
        _CACHE["nc"] = build_nc()
    nc = _CACHE["nc"]

    shared = {"winT": winT, "woutT": woutT, "g4": g4, "s4": s4, "su4": su4}
    in_maps = [
        {**shared,
         "h0": np.ascontiguousarray(h0[T * c:T * (c + 1), :]),
         "h0T": np.ascontiguousarray(h0T[:, T * c:T * (c + 1)])}
        for c in range(NCORES)
    ]
    res = run_bass_kernel_spmd(nc, in_maps, list(range(NCORES)), trace=_trace)
    out = np.concatenate(
        [np.asarray(res.results[c]["x_out"]) for c in range(NCORES)], axis=0)
    if _trace:
        _CACHE["last_result"] = res
    return out[None, :, :].astype(np.float32)
